# revision 20
# baseline (speedup 1.0000x reference)
"""DiversityLoss kernel for 8 Trainium2 NeuronCores.

Reference computes:
    loss = exp(mean(-D_img * D_noise))
where D_x[i,j] = (||x_i||^2 + ||x_j||^2 - 2 (X X^T)_ij) / d_x  for X in
{images, noises}.

The pairwise matrices never need to be materialized.  With
    a_i = ||img_i||^2, b_i = ||noise_i||^2, S1 = sum a, S2 = sum b,
    S3 = a.b, S4 = (Y^T a).(Y^T 1), S5 = (X^T b).(X^T 1), S6 = ||X^T Y||_F^2
the sum over all (i,j) of D_img*D_noise * (d_x*d_y) expands exactly to
    2*N*S3 + 2*S1*S2 - 4*S4 - 4*S5 + 4*S6
so   loss = exp(-(2*N*S3 + 2*S1*S2 - 4*S4 - 4*S5 + 4*S6) / (N^2 d_x d_y)).

Work split: every O(N*d) term (S1..S5) is computed exactly on the host in
fp32/fp64 BLAS -- they are cheap and exact there.  The single O(N*dx*dy)
term S6 = ||X^T Y||_F^2 runs on the device from fp8 data (validated ~1e-5
relative error on the final loss vs the fp32 reference).

Sharding: the feature (column) axis of the flattened images is split across
the 8 cores (1536 columns each); noises Y is replicated.  S6 splits into
per-core partial sums with no cross-core reduction; the host combines 4KB
of partials in fp64.

Per-core device program (one SPMD Bass program):
  - x arrives DoubleRow-interleaved [128, 16, 2, 1536] fp8; y8 = Y
    interleaved [128, 16, 2, 256] fp8.  All DMAs are emitted at the head
    of the program and only Sync/Scalar carry early instructions -- an
    engine with work at the program head gets its iram TENSOR_LOAD pulled
    into the startup barrier (~3.5us each), so Tensor/Vector work is
    emitted after the DMAs and their loads overlap the streaming.
  - 12 PSUM accumulation groups Z_j = X[:, j]^T @ Y over 16 DoubleRow
    pair-tiles, pair-outer so every group finishes one matmul after the
    last DMA chunk.  Two [128, 256] groups share each 2KB PSUM bank: the
    bank's single start=True matmul (even chunk, q=0) clears the bank's
    has_written bits, the odd chunk's first matmul (start=False) then
    overwrites its half because its bits are clear, and both accumulate
    thereafter.  stop=True only on the bank's last matmul.
  - drains: Z^2 -> S6 partials, banks 0-2 on ScalarE (Square activation
    with accum_out), banks 3-5 on VectorE (PSUM->SBUF copy then fused
    square-accumulate), running in parallel as banks complete.
  - two [128, 3] output DMAs on separate HWDGE rings so the second's
    completion latency hides behind the first's.
Output: f [128, 6] f32 split as fa/fb (partition x bank partials of S6).
"""

import os
import sys

import numpy as np

for _p in ("/opt/trn_rl_repo", "/root/.axon_site/_ro/trn_rl_repo"):
    if os.path.isdir(_p) and _p not in sys.path:
        sys.path.append(_p)

import ml_dtypes

N = 4096
DX = 12288
DY = 256
NCORES = 8
KC = DX // NCORES        # 1536 columns per core
T = N // 128             # 32 row tiles of 128
Q = T // 2               # 16 DoubleRow pair-tiles
KJ = KC // 128           # 12 stationary k-chunks per core
NB = KJ // 2             # 6 PSUM banks, 2 accumulation groups each
CHUNK_PAIRS = (1, 1, 1, 1, 1, 1, 2, 2, 2, 4)  # DMA chunking of the 16 pairs
NB_SCALAR = 4            # banks drained on ScalarE (rest on VectorE)
NWARM = 24               # PE clock warmup matmuls (emitted after the DMAs)

_PROG = None


def _build_program():
    from contextlib import ExitStack

    import concourse.bass as bass
    import concourse.tile as tile
    from concourse import bacc, mybir

    ts = bass.ts

    nc = bacc.Bacc(
        "TRN2",
        target_bir_lowering=False,
        debug=False,
        enable_asserts=False,
        num_devices=NCORES,
    )
    f32 = mybir.dt.float32
    bf16 = mybir.dt.bfloat16
    f8 = mybir.dt.float8e4
    DR = mybir.MatmulPerfMode.DoubleRow

    xd = nc.dram_tensor("x", [128, Q, 2, KC], f8, kind="ExternalInput").ap()
    y8d = nc.dram_tensor("y8", [128, Q, 2, DY], f8, kind="ExternalInput").ap()
    fa_out = nc.dram_tensor("fa", [128, NB_SCALAR], f32, kind="ExternalOutput").ap()
    fb_out = nc.dram_tensor("fb", [128, NB - NB_SCALAR], f32, kind="ExternalOutput").ap()

    MULT = mybir.AluOpType.mult
    SQ = mybir.ActivationFunctionType.Square

    bounds = []
    chunk_of = []
    q0 = 0
    for nq in CHUNK_PAIRS:
        bounds.append((q0, nq))
        chunk_of += [len(bounds) - 1] * nq
        q0 += nq
    assert q0 == Q

    with tile.TileContext(nc) as tc, ExitStack() as ctx:
        data = ctx.enter_context(tc.tile_pool(name="data", bufs=1))
        scr = ctx.enter_context(tc.tile_pool(name="scr", bufs=2))
        stats = ctx.enter_context(tc.tile_pool(name="stats", bufs=1))
        zpsum = ctx.enter_context(tc.tile_pool(name="zpsum", bufs=1, space="PSUM"))

        s6acc = stats.tile([128, NB], f32)

        zb = [zpsum.tile([128, 2 * DY], f32, tag=f"zb{i}", name=f"zb{i}") for i in range(NB)]

        # input DMAs first: y8 on the scalar HWDGE ring in two chunks (the
        # first is small so pair-0 matmuls are not gated on the whole 1MB);
        # x chunks in FIFO order on the sync ring so early pairs land first.
        Y8 = data.tile([128, Q, 2, DY], f8, name="Y8")
        nc.scalar.dma_start(Y8[:, 0:2, :, :], y8d[:, 0:2, :, :])
        nc.scalar.dma_start(Y8[:, 2:Q, :, :], y8d[:, 2:Q, :, :])
        xc = []
        for ci, (qq0, nq) in enumerate(bounds):
            xt = data.tile([128, nq, 2, KC], f8, tag=f"x{ci}", bufs=1, name=f"x{ci}")
            nc.sync.dma_start(xt[:], xd[:, qq0 : qq0 + nq, :, :])
            xc.append(xt)

        def x_pair(q):
            ci = chunk_of[q]
            return xc[ci][:, q - bounds[ci][0], :, :]

        # PE clock warmup: the HAM power manager holds the PE at half clock
        # until it sees ~5us of sustained matmul activity, so run dummy
        # matmuls while the first chunks stream in.  Emitted after the DMAs
        # so no extra engine program-load lands in the startup barrier.
        wst = stats.tile([128, 2, 128], f8)
        wmv = stats.tile([128, 2, 128], f8)
        wps = zpsum.tile([128, 128], f32, tag="wps", name="wps")
        nc.gpsimd.memset(wst[:], 1.0)
        nc.gpsimd.memset(wmv[:], 1.0)
        for _ in range(NWARM):
            nc.tensor.matmul(
                wps[:], lhsT=wst[:], rhs=wmv[:], perf_mode=DR, start=True, stop=True
            )

        # pair-outer accumulation: all 12 groups live simultaneously,
        # 2 per PSUM bank (j-ascending order, so each bank's two groups
        # finish back-to-back on the last pair and drains start early).
        for q in range(Q):
            for j in range(KJ):
                bk, half = divmod(j, 2)
                nc.tensor.matmul(
                    zb[bk][:, half * DY : (half + 1) * DY],
                    lhsT=x_pair(q)[:, :, ts(j, 128)],
                    rhs=Y8[:, q, :, :],
                    perf_mode=DR,
                    start=(q == 0 and half == 0),
                    stop=(q == Q - 1 and half == 1),
                )

        # drains, split across ScalarE and VectorE so they run in parallel
        # as banks complete.  ScalarE squares PSUM directly; VectorE must
        # copy to SBUF first (one PSUM operand per instruction).
        for bk in range(NB_SCALAR):
            zsq = scr.tile([128, 2 * DY], bf16, tag="zsq", name="zsq")
            nc.scalar.activation(
                zsq[:], zb[bk][:], SQ, accum_out=s6acc[:, bk : bk + 1]
            )
        for bk in range(NB_SCALAR, NB):
            zc = scr.tile([128, 2 * DY], f32, tag="zc", name="zc")
            nc.vector.tensor_copy(zc[:], zb[bk][:])
            vsq = scr.tile([128, 2 * DY], bf16, tag="vsq", name="vsq")
            nc.vector.scalar_tensor_tensor(
                out=vsq[:],
                in0=zc[:],
                scalar=1.0,
                in1=zc[:],
                op0=MULT,
                op1=MULT,
                accum_out=s6acc[:, bk : bk + 1],
            )

        # two output DMAs on separate rings; each waits only its engine's
        # drains, so the completion latencies overlap.
        nc.scalar.dma_start(fa_out, s6acc[:, 0:NB_SCALAR])
        nc.sync.dma_start(fb_out, s6acc[:, NB_SCALAR:NB])

    nc.compile()
    return nc


def _get_program():
    global _PROG
    if _PROG is None:
        _PROG = _build_program()
    return _PROG


_LAST_RESULTS = None


def kernel(noises: np.ndarray, images: np.ndarray) -> np.ndarray:
    from concourse import bass_utils

    global _LAST_RESULTS

    nc = _get_program()

    X = np.ascontiguousarray(images, dtype=np.float32).reshape(N, -1)
    Y = np.ascontiguousarray(noises, dtype=np.float32)

    # host-exact small terms (all O(N*d))
    a = np.einsum("ij,ij->i", X, X, dtype=np.float32)
    b = np.einsum("ij,ij->i", Y, Y, dtype=np.float32)
    ones = np.ones(N, dtype=np.float32)
    H = Y.T @ np.stack([a, ones], axis=1)            # [DY, 2] fp32 sgemm
    G = np.stack([b, ones], axis=1).T @ X            # [2, DX] fp32 sgemm
    a64 = a.astype(np.float64)
    b64 = b.astype(np.float64)
    S1 = a64.sum()
    S2 = b64.sum()
    S3 = a64 @ b64
    S4 = float(H[:, 0].astype(np.float64) @ H[:, 1].astype(np.float64))
    S5 = float(G[0].astype(np.float64) @ G[1].astype(np.float64))

    # device inputs: fp8, DoubleRow pair-interleaved partition-major
    x8 = X.astype(ml_dtypes.float8_e4m3)
    y8 = np.ascontiguousarray(
        Y.astype(ml_dtypes.float8_e4m3).reshape(Q, 2, 128, DY).transpose(2, 0, 1, 3)
    )
    in_maps = []
    for c in range(NCORES):
        xcore = np.ascontiguousarray(
            x8[:, c * KC : (c + 1) * KC].reshape(Q, 2, 128, KC).transpose(2, 0, 1, 3)
        )
        in_maps.append({"x": xcore, "y8": y8})

    res = bass_utils.run_bass_kernel_spmd(nc, in_maps, core_ids=list(range(NCORES)))
    _LAST_RESULTS = res

    S6 = 0.0
    for c in range(NCORES):
        S6 += np.asarray(res.results[c]["fa"], dtype=np.float64).sum()
        S6 += np.asarray(res.results[c]["fb"], dtype=np.float64).sum()

    num = 2.0 * N * S3 + 2.0 * S1 * S2 - 4.0 * S4 - 4.0 * S5 + 4.0 * S6
    mean = num / (float(N) * N * DX * DY)
    return np.asarray(np.exp(-mean), dtype=np.float32)


# revision 22
# speedup vs baseline: 1.0403x; 1.0403x over previous
"""DiversityLoss kernel for 8 Trainium2 NeuronCores.

Reference computes:
    loss = exp(mean(-D_img * D_noise))
where D_x[i,j] = (||x_i||^2 + ||x_j||^2 - 2 (X X^T)_ij) / d_x  for X in
{images, noises}.

The pairwise matrices never need to be materialized.  With
    a_i = ||img_i||^2, b_i = ||noise_i||^2, S1 = sum a, S2 = sum b,
    S3 = a.b, S4 = (Y^T a).(Y^T 1), S5 = (X^T b).(X^T 1), S6 = ||X^T Y||_F^2
the sum over all (i,j) of D_img*D_noise * (d_x*d_y) expands exactly to
    2*N*S3 + 2*S1*S2 - 4*S4 - 4*S5 + 4*S6
so   loss = exp(-(2*N*S3 + 2*S1*S2 - 4*S4 - 4*S5 + 4*S6) / (N^2 d_x d_y)).

Work split: every O(N*d) term (S1..S5) is computed exactly on the host in
fp32/fp64 BLAS -- they are cheap and exact there.  The single O(N*dx*dy)
term S6 = ||X^T Y||_F^2 runs on the device from fp8 data (validated ~1e-5
relative error on the final loss vs the fp32 reference).

Sharding: the feature (column) axis of the flattened images is split across
the 8 cores (1536 columns each); noises Y is replicated.  S6 splits into
per-core partial sums with no cross-core reduction; the host combines 4KB
of partials in fp64.

Per-core device program (one SPMD Bass program):
  - x arrives DoubleRow-interleaved [128, 16, 2, 1536] fp8; y8 = Y
    interleaved [128, 16, 2, 256] fp8.  All DMAs are emitted at the head
    of the program and only Sync/Scalar carry early instructions -- an
    engine with work at the program head gets its iram TENSOR_LOAD pulled
    into the startup barrier (~3.5us each), so Tensor/Vector work is
    emitted after the DMAs and their loads overlap the streaming.
  - 12 PSUM accumulation groups Z_j = X[:, j]^T @ Y over 16 DoubleRow
    pair-tiles, pair-outer so every group finishes one matmul after the
    last DMA chunk.  Two [128, 256] groups share each 2KB PSUM bank: the
    bank's single start=True matmul (even chunk, q=0) clears the bank's
    has_written bits, the odd chunk's first matmul (start=False) then
    overwrites its half because its bits are clear, and both accumulate
    thereafter.  stop=True only on the bank's last matmul.
  - drains: Z^2 -> S6 partials, banks 0-2 on ScalarE (Square activation
    with accum_out), banks 3-5 on VectorE (PSUM->SBUF copy then fused
    square-accumulate), running in parallel as banks complete.
  - two [128, 3] output DMAs on separate HWDGE rings so the second's
    completion latency hides behind the first's.
Output: f [128, 6] f32 split as fa/fb (partition x bank partials of S6).
"""

import os
import sys

import numpy as np

for _p in ("/opt/trn_rl_repo", "/root/.axon_site/_ro/trn_rl_repo"):
    if os.path.isdir(_p) and _p not in sys.path:
        sys.path.append(_p)

import ml_dtypes

N = 4096
DX = 12288
DY = 256
NCORES = 8
KC = DX // NCORES        # 1536 columns per core
T = N // 128             # 32 row tiles of 128
Q = T // 2               # 16 DoubleRow pair-tiles
KJ = KC // 128           # 12 stationary k-chunks per core
NB = KJ // 2             # 6 PSUM banks, 2 accumulation groups each
CHUNK_PAIRS = (1, 1, 1, 1, 1, 1, 2, 2, 2, 4)  # DMA chunking of the 16 pairs
NB_SCALAR = 4            # banks drained on ScalarE (rest on VectorE)
NWARM = 7                # PE clock warmup matmuls (emitted after the DMAs)

_PROG = None


def _build_program():
    from contextlib import ExitStack

    import concourse.bass as bass
    import concourse.tile as tile
    from concourse import bacc, mybir

    ts = bass.ts

    nc = bacc.Bacc(
        "TRN2",
        target_bir_lowering=False,
        debug=False,
        enable_asserts=False,
        num_devices=NCORES,
    )
    f32 = mybir.dt.float32
    bf16 = mybir.dt.bfloat16
    f8 = mybir.dt.float8e4
    DR = mybir.MatmulPerfMode.DoubleRow

    xd = nc.dram_tensor("x", [128, Q, 2, KC], f8, kind="ExternalInput").ap()
    y8d = nc.dram_tensor("y8", [128, Q, 2, DY], f8, kind="ExternalInput").ap()
    fa_out = nc.dram_tensor("fa", [128, NB_SCALAR], f32, kind="ExternalOutput").ap()
    fb_out = nc.dram_tensor("fb", [128, NB - NB_SCALAR], f32, kind="ExternalOutput").ap()

    MULT = mybir.AluOpType.mult
    SQ = mybir.ActivationFunctionType.Square

    bounds = []
    chunk_of = []
    q0 = 0
    for nq in CHUNK_PAIRS:
        bounds.append((q0, nq))
        chunk_of += [len(bounds) - 1] * nq
        q0 += nq
    assert q0 == Q

    with tile.TileContext(nc) as tc, ExitStack() as ctx:
        data = ctx.enter_context(tc.tile_pool(name="data", bufs=1))
        scr = ctx.enter_context(tc.tile_pool(name="scr", bufs=2))
        stats = ctx.enter_context(tc.tile_pool(name="stats", bufs=1))
        zpsum = ctx.enter_context(tc.tile_pool(name="zpsum", bufs=1, space="PSUM"))

        s6acc = stats.tile([128, NB], f32)

        zb = [zpsum.tile([128, 2 * DY], f32, tag=f"zb{i}", name=f"zb{i}") for i in range(NB)]

        # input DMAs first: y8 on the scalar HWDGE ring in two chunks (the
        # first is small so pair-0 matmuls are not gated on the whole 1MB);
        # x chunks in FIFO order on the sync ring so early pairs land first.
        Y8 = data.tile([128, Q, 2, DY], f8, name="Y8")
        nc.scalar.dma_start(Y8[:, 0:2, :, :], y8d[:, 0:2, :, :])
        nc.scalar.dma_start(Y8[:, 2:Q, :, :], y8d[:, 2:Q, :, :])
        xc = []
        for ci, (qq0, nq) in enumerate(bounds):
            xt = data.tile([128, nq, 2, KC], f8, tag=f"x{ci}", bufs=1, name=f"x{ci}")
            nc.sync.dma_start(xt[:], xd[:, qq0 : qq0 + nq, :, :])
            xc.append(xt)

        def x_pair(q):
            ci = chunk_of[q]
            return xc[ci][:, q - bounds[ci][0], :, :]

        # PE clock warmup: the HAM power manager holds the PE at half clock
        # until it sees ~5us of sustained matmul activity, so run dummy
        # matmuls while the first chunks stream in.  Emitted after the DMAs
        # so no extra engine program-load lands in the startup barrier.
        wst = stats.tile([128, 2, 128], f8)
        wmv = stats.tile([128, 2, 512], f8)
        wps = zpsum.tile([128, 512], f32, tag="wps", name="wps")
        nc.gpsimd.memset(wst[:], 1.0)
        nc.gpsimd.memset(wmv[:], 1.0)
        for _ in range(NWARM):
            nc.tensor.matmul(
                wps[:], lhsT=wst[:], rhs=wmv[:], perf_mode=DR, start=True, stop=True
            )

        # pair-outer accumulation: all 12 groups live simultaneously,
        # 2 per PSUM bank (j-ascending order, so each bank's two groups
        # finish back-to-back on the last pair and drains start early).
        for q in range(Q):
            for j in range(KJ):
                bk, half = divmod(j, 2)
                nc.tensor.matmul(
                    zb[bk][:, half * DY : (half + 1) * DY],
                    lhsT=x_pair(q)[:, :, ts(j, 128)],
                    rhs=Y8[:, q, :, :],
                    perf_mode=DR,
                    start=(q == 0 and half == 0),
                    stop=(q == Q - 1 and half == 1),
                )

        # drains, split across ScalarE and VectorE so they run in parallel
        # as banks complete.  ScalarE squares PSUM directly; VectorE must
        # copy to SBUF first (one PSUM operand per instruction).
        for bk in range(NB_SCALAR):
            zsq = scr.tile([128, 2 * DY], bf16, tag="zsq", name="zsq")
            nc.scalar.activation(
                zsq[:], zb[bk][:], SQ, accum_out=s6acc[:, bk : bk + 1]
            )
        for bk in range(NB_SCALAR, NB):
            zc = scr.tile([128, 2 * DY], f32, tag="zc", name="zc")
            nc.vector.tensor_copy(zc[:], zb[bk][:])
            vsq = scr.tile([128, 2 * DY], bf16, tag="vsq", name="vsq")
            nc.vector.scalar_tensor_tensor(
                out=vsq[:],
                in0=zc[:],
                scalar=1.0,
                in1=zc[:],
                op0=MULT,
                op1=MULT,
                accum_out=s6acc[:, bk : bk + 1],
            )

        # two output DMAs on separate rings; each waits only its engine's
        # drains, so the completion latencies overlap.
        nc.scalar.dma_start(fa_out, s6acc[:, 0:NB_SCALAR])
        nc.sync.dma_start(fb_out, s6acc[:, NB_SCALAR:NB])

    nc.compile()
    return nc


def _get_program():
    global _PROG
    if _PROG is None:
        _PROG = _build_program()
    return _PROG


_LAST_RESULTS = None


def kernel(noises: np.ndarray, images: np.ndarray) -> np.ndarray:
    from concourse import bass_utils

    global _LAST_RESULTS

    nc = _get_program()

    X = np.ascontiguousarray(images, dtype=np.float32).reshape(N, -1)
    Y = np.ascontiguousarray(noises, dtype=np.float32)

    # host-exact small terms (all O(N*d))
    a = np.einsum("ij,ij->i", X, X, dtype=np.float32)
    b = np.einsum("ij,ij->i", Y, Y, dtype=np.float32)
    ones = np.ones(N, dtype=np.float32)
    H = Y.T @ np.stack([a, ones], axis=1)            # [DY, 2] fp32 sgemm
    G = np.stack([b, ones], axis=1).T @ X            # [2, DX] fp32 sgemm
    a64 = a.astype(np.float64)
    b64 = b.astype(np.float64)
    S1 = a64.sum()
    S2 = b64.sum()
    S3 = a64 @ b64
    S4 = float(H[:, 0].astype(np.float64) @ H[:, 1].astype(np.float64))
    S5 = float(G[0].astype(np.float64) @ G[1].astype(np.float64))

    # device inputs: fp8, DoubleRow pair-interleaved partition-major
    x8 = X.astype(ml_dtypes.float8_e4m3)
    y8 = np.ascontiguousarray(
        Y.astype(ml_dtypes.float8_e4m3).reshape(Q, 2, 128, DY).transpose(2, 0, 1, 3)
    )
    in_maps = []
    for c in range(NCORES):
        xcore = np.ascontiguousarray(
            x8[:, c * KC : (c + 1) * KC].reshape(Q, 2, 128, KC).transpose(2, 0, 1, 3)
        )
        in_maps.append({"x": xcore, "y8": y8})

    res = bass_utils.run_bass_kernel_spmd(nc, in_maps, core_ids=list(range(NCORES)))
    _LAST_RESULTS = res

    S6 = 0.0
    for c in range(NCORES):
        S6 += np.asarray(res.results[c]["fa"], dtype=np.float64).sum()
        S6 += np.asarray(res.results[c]["fb"], dtype=np.float64).sum()

    num = 2.0 * N * S3 + 2.0 * S1 * S2 - 4.0 * S4 - 4.0 * S5 + 4.0 * S6
    mean = num / (float(N) * N * DX * DY)
    return np.asarray(np.exp(-mean), dtype=np.float32)


# revision 23
# speedup vs baseline: 1.0541x; 1.0133x over previous
"""DiversityLoss kernel for 8 Trainium2 NeuronCores.

Reference computes:
    loss = exp(mean(-D_img * D_noise))
where D_x[i,j] = (||x_i||^2 + ||x_j||^2 - 2 (X X^T)_ij) / d_x  for X in
{images, noises}.

The pairwise matrices never need to be materialized.  With
    a_i = ||img_i||^2, b_i = ||noise_i||^2, S1 = sum a, S2 = sum b,
    S3 = a.b, S4 = (Y^T a).(Y^T 1), S5 = (X^T b).(X^T 1), S6 = ||X^T Y||_F^2
the sum over all (i,j) of D_img*D_noise * (d_x*d_y) expands exactly to
    2*N*S3 + 2*S1*S2 - 4*S4 - 4*S5 + 4*S6
so   loss = exp(-(2*N*S3 + 2*S1*S2 - 4*S4 - 4*S5 + 4*S6) / (N^2 d_x d_y)).

Work split: every O(N*d) term (S1..S5) is computed exactly on the host in
fp32/fp64 BLAS -- they are cheap and exact there.  The single O(N*dx*dy)
term S6 = ||X^T Y||_F^2 runs on the device from fp8 data (validated ~1e-5
relative error on the final loss vs the fp32 reference).

Sharding: the feature (column) axis of the flattened images is split across
the 8 cores (1536 columns each); noises Y is replicated.  S6 splits into
per-core partial sums with no cross-core reduction; the host combines 4KB
of partials in fp64.

Per-core device program (one SPMD Bass program):
  - x arrives DoubleRow-interleaved [128, 16, 2, 1536] fp8; y8 = Y
    interleaved [128, 16, 2, 256] fp8.  All DMAs are emitted at the head
    of the program and only Sync/Scalar carry early instructions -- an
    engine with work at the program head gets its iram TENSOR_LOAD pulled
    into the startup barrier (~3.5us each), so Tensor/Vector work is
    emitted after the DMAs and their loads overlap the streaming.
  - 12 PSUM accumulation groups Z_j = X[:, j]^T @ Y over 16 DoubleRow
    pair-tiles, pair-outer so every group finishes one matmul after the
    last DMA chunk.  Two [128, 256] groups share each 2KB PSUM bank: the
    bank's single start=True matmul (even chunk, q=0) clears the bank's
    has_written bits, the odd chunk's first matmul (start=False) then
    overwrites its half because its bits are clear, and both accumulate
    thereafter.  stop=True only on the bank's last matmul.
  - drains: Z^2 -> S6 partials, banks 0-2 on ScalarE (Square activation
    with accum_out), banks 3-5 on VectorE (PSUM->SBUF copy then fused
    square-accumulate), running in parallel as banks complete.
  - two [128, 3] output DMAs on separate HWDGE rings so the second's
    completion latency hides behind the first's.
Output: f [128, 6] f32 split as fa/fb (partition x bank partials of S6).
"""

import os
import sys

import numpy as np

for _p in ("/opt/trn_rl_repo", "/root/.axon_site/_ro/trn_rl_repo"):
    if os.path.isdir(_p) and _p not in sys.path:
        sys.path.append(_p)

import ml_dtypes

N = 4096
DX = 12288
DY = 256
NCORES = 8
KC = DX // NCORES        # 1536 columns per core
T = N // 128             # 32 row tiles of 128
Q = T // 2               # 16 DoubleRow pair-tiles
KJ = KC // 128           # 12 stationary k-chunks per core
NB = KJ // 2             # 6 PSUM banks, 2 accumulation groups each
CHUNK_PAIRS = (1, 1, 1, 1, 1, 1, 2, 2, 2, 4)  # DMA chunking of the 16 pairs
NB_SCALAR = 4            # banks drained on ScalarE (rest on VectorE)
NWARM = 7                # PE clock warmup matmuls (emitted after the DMAs)

_PROG = None


def _build_program():
    from contextlib import ExitStack

    import concourse.bass as bass
    import concourse.tile as tile
    from concourse import bacc, mybir

    ts = bass.ts

    nc = bacc.Bacc(
        "TRN2",
        target_bir_lowering=False,
        debug=False,
        enable_asserts=False,
        num_devices=NCORES,
    )
    f32 = mybir.dt.float32
    bf16 = mybir.dt.bfloat16
    f8 = mybir.dt.float8e4
    DR = mybir.MatmulPerfMode.DoubleRow

    xd = nc.dram_tensor("x", [128, Q, 2, KC], f8, kind="ExternalInput").ap()
    y8d = nc.dram_tensor("y8", [128, Q, 2, DY], f8, kind="ExternalInput").ap()
    fa_out = nc.dram_tensor("fa", [128, NB_SCALAR], f32, kind="ExternalOutput").ap()
    fb_out = nc.dram_tensor("fb", [128, NB - NB_SCALAR], f32, kind="ExternalOutput").ap()

    MULT = mybir.AluOpType.mult
    SQ = mybir.ActivationFunctionType.Square

    bounds = []
    chunk_of = []
    q0 = 0
    for nq in CHUNK_PAIRS:
        bounds.append((q0, nq))
        chunk_of += [len(bounds) - 1] * nq
        q0 += nq
    assert q0 == Q

    with tile.TileContext(nc) as tc, ExitStack() as ctx:
        data = ctx.enter_context(tc.tile_pool(name="data", bufs=1))
        scr = ctx.enter_context(tc.tile_pool(name="scr", bufs=2))
        stats = ctx.enter_context(tc.tile_pool(name="stats", bufs=1))
        zpsum = ctx.enter_context(tc.tile_pool(name="zpsum", bufs=1, space="PSUM"))

        s6acc = stats.tile([128, NB], f32)

        zb = [zpsum.tile([128, 2 * DY], f32, tag=f"zb{i}", name=f"zb{i}") for i in range(NB)]

        # input DMAs first: y8 on the scalar HWDGE ring in two chunks (the
        # first is small so pair-0 matmuls are not gated on the whole 1MB);
        # x chunks in FIFO order on the sync ring so early pairs land first.
        Y8 = data.tile([128, Q, 2, DY], f8, name="Y8")
        nc.scalar.dma_start(Y8[:, 0:2, :, :], y8d[:, 0:2, :, :])
        nc.scalar.dma_start(Y8[:, 2:Q, :, :], y8d[:, 2:Q, :, :])
        xc = []
        for ci, (qq0, nq) in enumerate(bounds):
            xt = data.tile([128, nq, 2, KC], f8, tag=f"x{ci}", bufs=1, name=f"x{ci}")
            nc.sync.dma_start(xt[:], xd[:, qq0 : qq0 + nq, :, :])
            xc.append(xt)

        def x_pair(q):
            ci = chunk_of[q]
            return xc[ci][:, q - bounds[ci][0], :, :]

        # PE clock warmup: the HAM power manager holds the PE at half clock
        # until it sees ~5us of sustained matmul activity, so run dummy
        # matmuls while the first chunks stream in.  Emitted after the DMAs
        # so no extra engine program-load lands in the startup barrier.
        wst = stats.tile([128, 2, 128], f8)
        wmv = stats.tile([128, 2, 512], f8)
        wps = zpsum.tile([128, 512], f32, tag="wps", name="wps")
        nc.gpsimd.memset(wst[:], 1.0)
        nc.gpsimd.memset(wmv[:], 1.0)
        for _ in range(NWARM):
            nc.tensor.matmul(
                wps[:], lhsT=wst[:], rhs=wmv[:], perf_mode=DR, start=True, stop=True
            )

        # pair-outer accumulation: all 12 groups live simultaneously,
        # 2 per PSUM bank (j-ascending order, so each bank's two groups
        # finish back-to-back on the last pair and drains start early).
        for q in range(Q):
            for j in range(KJ):
                bk, half = divmod(j, 2)
                nc.tensor.matmul(
                    zb[bk][:, half * DY : (half + 1) * DY],
                    lhsT=x_pair(q)[:, :, ts(j, 128)],
                    rhs=Y8[:, q, :, :],
                    perf_mode=DR,
                    start=(q == 0 and half == 0),
                    stop=(q == Q - 1 and half == 1),
                )

        # drains, split across VectorE and ScalarE so they run in parallel
        # as banks complete.  VectorE is ~2x slower per bank (it must copy
        # to SBUF first -- one PSUM operand per instruction), so it takes
        # the two earliest-finishing banks; ScalarE squares PSUM directly
        # and takes the remaining four.
        NB_VEC = NB - NB_SCALAR
        for bk in range(NB_VEC):
            zc = scr.tile([128, 2 * DY], f32, tag="zc", name="zc")
            nc.vector.tensor_copy(zc[:], zb[bk][:])
            vsq = scr.tile([128, 2 * DY], bf16, tag="vsq", name="vsq")
            nc.vector.scalar_tensor_tensor(
                out=vsq[:],
                in0=zc[:],
                scalar=1.0,
                in1=zc[:],
                op0=MULT,
                op1=MULT,
                accum_out=s6acc[:, bk : bk + 1],
            )
        for bk in range(NB_VEC, NB):
            zsq = scr.tile([128, 2 * DY], bf16, tag="zsq", name="zsq")
            nc.scalar.activation(
                zsq[:], zb[bk][:], SQ, accum_out=s6acc[:, bk : bk + 1]
            )

        # two output DMAs on the idle sync ring, in drain-completion order
        # so the second's completion latency hides behind the first's.
        nc.sync.dma_start(fb_out, s6acc[:, 0:NB_VEC])
        nc.sync.dma_start(fa_out, s6acc[:, NB_VEC:NB])

    nc.compile()
    return nc


def _get_program():
    global _PROG
    if _PROG is None:
        _PROG = _build_program()
    return _PROG


_LAST_RESULTS = None


def kernel(noises: np.ndarray, images: np.ndarray) -> np.ndarray:
    from concourse import bass_utils

    global _LAST_RESULTS

    nc = _get_program()

    X = np.ascontiguousarray(images, dtype=np.float32).reshape(N, -1)
    Y = np.ascontiguousarray(noises, dtype=np.float32)

    # host-exact small terms (all O(N*d))
    a = np.einsum("ij,ij->i", X, X, dtype=np.float32)
    b = np.einsum("ij,ij->i", Y, Y, dtype=np.float32)
    ones = np.ones(N, dtype=np.float32)
    H = Y.T @ np.stack([a, ones], axis=1)            # [DY, 2] fp32 sgemm
    G = np.stack([b, ones], axis=1).T @ X            # [2, DX] fp32 sgemm
    a64 = a.astype(np.float64)
    b64 = b.astype(np.float64)
    S1 = a64.sum()
    S2 = b64.sum()
    S3 = a64 @ b64
    S4 = float(H[:, 0].astype(np.float64) @ H[:, 1].astype(np.float64))
    S5 = float(G[0].astype(np.float64) @ G[1].astype(np.float64))

    # device inputs: fp8, DoubleRow pair-interleaved partition-major
    x8 = X.astype(ml_dtypes.float8_e4m3)
    y8 = np.ascontiguousarray(
        Y.astype(ml_dtypes.float8_e4m3).reshape(Q, 2, 128, DY).transpose(2, 0, 1, 3)
    )
    in_maps = []
    for c in range(NCORES):
        xcore = np.ascontiguousarray(
            x8[:, c * KC : (c + 1) * KC].reshape(Q, 2, 128, KC).transpose(2, 0, 1, 3)
        )
        in_maps.append({"x": xcore, "y8": y8})

    res = bass_utils.run_bass_kernel_spmd(nc, in_maps, core_ids=list(range(NCORES)))
    _LAST_RESULTS = res

    S6 = 0.0
    for c in range(NCORES):
        S6 += np.asarray(res.results[c]["fa"], dtype=np.float64).sum()
        S6 += np.asarray(res.results[c]["fb"], dtype=np.float64).sum()

    num = 2.0 * N * S3 + 2.0 * S1 * S2 - 4.0 * S4 - 4.0 * S5 + 4.0 * S6
    mean = num / (float(N) * N * DX * DY)
    return np.asarray(np.exp(-mean), dtype=np.float32)


# revision 26
# speedup vs baseline: 1.0723x; 1.0173x over previous
"""DiversityLoss kernel for 8 Trainium2 NeuronCores.

Reference computes:
    loss = exp(mean(-D_img * D_noise))
where D_x[i,j] = (||x_i||^2 + ||x_j||^2 - 2 (X X^T)_ij) / d_x  for X in
{images, noises}.

The pairwise matrices never need to be materialized.  With
    a_i = ||img_i||^2, b_i = ||noise_i||^2, S1 = sum a, S2 = sum b,
    S3 = a.b, S4 = (Y^T a).(Y^T 1), S5 = (X^T b).(X^T 1), S6 = ||X^T Y||_F^2
the sum over all (i,j) of D_img*D_noise * (d_x*d_y) expands exactly to
    2*N*S3 + 2*S1*S2 - 4*S4 - 4*S5 + 4*S6
so   loss = exp(-(2*N*S3 + 2*S1*S2 - 4*S4 - 4*S5 + 4*S6) / (N^2 d_x d_y)).

Work split: every O(N*d) term (S1..S5) is computed exactly on the host in
fp32/fp64 BLAS -- they are cheap and exact there.  The single O(N*dx*dy)
term S6 = ||X^T Y||_F^2 runs on the device from fp8 data (validated ~1e-5
relative error on the final loss vs the fp32 reference).

Sharding: the feature (column) axis of the flattened images is split across
the 8 cores (1536 columns each); noises Y is replicated.  S6 splits into
per-core partial sums with no cross-core reduction; the host combines 4KB
of partials in fp64.

Per-core device program (one SPMD Bass program):
  - x arrives DoubleRow-interleaved [128, 16, 2, 1536] fp8; y8 = Y
    interleaved [128, 16, 2, 256] fp8.  All DMAs are emitted at the head
    of the program and only Sync/Scalar carry early instructions -- an
    engine with work at the program head gets its iram TENSOR_LOAD pulled
    into the startup barrier (~3.5us each), so Tensor/Vector work is
    emitted after the DMAs and their loads overlap the streaming.
  - 12 PSUM accumulation groups Z_j = X[:, j]^T @ Y over 16 DoubleRow
    pair-tiles, pair-outer so every group finishes one matmul after the
    last DMA chunk.  Two [128, 256] groups share each 2KB PSUM bank: the
    bank's single start=True matmul (even chunk, q=0) clears the bank's
    has_written bits, the odd chunk's first matmul (start=False) then
    overwrites its half because its bits are clear, and both accumulate
    thereafter.  stop=True only on the bank's last matmul.
  - drains: Z^2 -> S6 partials, banks 0-2 on ScalarE (Square activation
    with accum_out), banks 3-5 on VectorE (PSUM->SBUF copy then fused
    square-accumulate), running in parallel as banks complete.
  - two [128, 3] output DMAs on separate HWDGE rings so the second's
    completion latency hides behind the first's.
Output: f [128, 6] f32 split as fa/fb (partition x bank partials of S6).
"""

import os
import sys

import numpy as np

for _p in ("/opt/trn_rl_repo", "/root/.axon_site/_ro/trn_rl_repo"):
    if os.path.isdir(_p) and _p not in sys.path:
        sys.path.append(_p)

import ml_dtypes

N = 4096
DX = 12288
DY = 256
NCORES = 8
KC = DX // NCORES        # 1536 columns per core
T = N // 128             # 32 row tiles of 128
Q = T // 2               # 16 DoubleRow pair-tiles
KJ = KC // 128           # 12 stationary k-chunks per core
NB = KJ // 2             # 6 PSUM banks, 2 accumulation groups each
CHUNK_PAIRS = (1, 1, 1, 1, 1, 1, 2, 4, 4)  # DMA chunking of the 16 pairs
X0_SPLIT = 512           # pair-0 ships as 0:512 / 512:1536 column sub-chunks so
                         # the first matmuls' DMA-completion receipt comes early
NB_SCALAR = 4            # banks drained on ScalarE (rest on VectorE)
NWARM = 3                # PE clock warmup matmuls (emitted after the DMAs)

_PROG = None


def _build_program():
    from contextlib import ExitStack

    import concourse.bass as bass
    import concourse.tile as tile
    from concourse import bacc, mybir

    ts = bass.ts

    nc = bacc.Bacc(
        "TRN2",
        target_bir_lowering=False,
        debug=False,
        enable_asserts=False,
        num_devices=NCORES,
    )
    f32 = mybir.dt.float32
    bf16 = mybir.dt.bfloat16
    f8 = mybir.dt.float8e4
    DR = mybir.MatmulPerfMode.DoubleRow

    xd = nc.dram_tensor("x", [128, Q, 2, KC], f8, kind="ExternalInput").ap()
    y8d = nc.dram_tensor("y8", [128, Q, 2, DY], f8, kind="ExternalInput").ap()
    fa_out = nc.dram_tensor("fa", [128, NB_SCALAR], f32, kind="ExternalOutput").ap()
    fb_out = nc.dram_tensor("fb", [128, NB - NB_SCALAR], f32, kind="ExternalOutput").ap()

    MULT = mybir.AluOpType.mult
    SQ = mybir.ActivationFunctionType.Square

    bounds = []
    chunk_of = []
    q0 = 0
    for nq in CHUNK_PAIRS:
        bounds.append((q0, nq))
        chunk_of += [len(bounds) - 1] * nq
        q0 += nq
    assert q0 == Q

    with tile.TileContext(nc) as tc, ExitStack() as ctx:
        data = ctx.enter_context(tc.tile_pool(name="data", bufs=1))
        scr = ctx.enter_context(tc.tile_pool(name="scr", bufs=2))
        stats = ctx.enter_context(tc.tile_pool(name="stats", bufs=1))
        zpsum = ctx.enter_context(tc.tile_pool(name="zpsum", bufs=1, space="PSUM"))

        s6acc = stats.tile([128, NB], f32)

        zb = [zpsum.tile([128, 2 * DY], f32, tag=f"zb{i}", name=f"zb{i}") for i in range(NB)]

        # input DMAs first: y8 on the scalar HWDGE ring in two chunks (the
        # first is small so pair-0 matmuls are not gated on the whole 1MB);
        # x chunks in FIFO order on the sync ring so early pairs land first.
        Y8 = data.tile([128, Q, 2, DY], f8, name="Y8")
        nc.scalar.dma_start(Y8[:, 0:2, :, :], y8d[:, 0:2, :, :])
        nc.scalar.dma_start(Y8[:, 2:Q, :, :], y8d[:, 2:Q, :, :])
        x0a = data.tile([128, 1, 2, X0_SPLIT], f8, tag="x0a", bufs=1, name="x0a")
        x0b = data.tile([128, 1, 2, KC - X0_SPLIT], f8, tag="x0b", bufs=1, name="x0b")
        nc.sync.dma_start(x0a[:], xd[:, 0:1, :, 0:X0_SPLIT])
        nc.sync.dma_start(x0b[:], xd[:, 0:1, :, X0_SPLIT:KC])
        xc = []
        for ci, (qq0, nq) in enumerate(bounds):
            if ci == 0:
                xc.append(None)
                continue
            xt = data.tile([128, nq, 2, KC], f8, tag=f"x{ci}", bufs=1, name=f"x{ci}")
            nc.sync.dma_start(xt[:], xd[:, qq0 : qq0 + nq, :, :])
            xc.append(xt)

        def x_lhsT(q, j):
            if q == 0:
                c0 = j * 128
                if c0 < X0_SPLIT:
                    return x0a[:, 0, :, c0 : c0 + 128]
                return x0b[:, 0, :, c0 - X0_SPLIT : c0 - X0_SPLIT + 128]
            ci = chunk_of[q]
            return xc[ci][:, q - bounds[ci][0], :, ts(j, 128)]

        # PE clock warmup: the HAM power manager holds the PE at half clock
        # until it sees ~5us of sustained matmul activity, so run dummy
        # matmuls while the first chunks stream in.  Emitted after the DMAs
        # so no extra engine program-load lands in the startup barrier.
        wst = stats.tile([128, 2, 128], f8)
        wmv = stats.tile([128, 2, 512], f8)
        wps = zpsum.tile([128, 512], f32, tag="wps", name="wps")
        nc.gpsimd.memset(wst[:], 1.0)
        nc.gpsimd.memset(wmv[:], 1.0)
        for _ in range(NWARM):
            nc.tensor.matmul(
                wps[:], lhsT=wst[:], rhs=wmv[:], perf_mode=DR, start=True, stop=True
            )

        # pair-outer accumulation: all 12 groups live simultaneously,
        # 2 per PSUM bank (j-ascending order, so each bank's two groups
        # finish back-to-back on the last pair and drains start early).
        for q in range(Q):
            for j in range(KJ):
                bk, half = divmod(j, 2)
                nc.tensor.matmul(
                    zb[bk][:, half * DY : (half + 1) * DY],
                    lhsT=x_lhsT(q, j),
                    rhs=Y8[:, q, :, :],
                    perf_mode=DR,
                    start=(q == 0 and half == 0),
                    stop=(q == Q - 1 and half == 1),
                )

        # drains, split across VectorE and ScalarE so they run in parallel
        # as banks complete.  VectorE is ~2x slower per bank (it must copy
        # to SBUF first -- one PSUM operand per instruction), so it takes
        # the two earliest-finishing banks; ScalarE squares PSUM directly
        # and takes the remaining four.
        NB_VEC = NB - NB_SCALAR
        for bk in range(NB_VEC):
            zc = scr.tile([128, 2 * DY], f32, tag="zc", name="zc")
            nc.vector.tensor_copy(zc[:], zb[bk][:])
            vsq = scr.tile([128, 2 * DY], bf16, tag="vsq", name="vsq")
            nc.vector.scalar_tensor_tensor(
                out=vsq[:],
                in0=zc[:],
                scalar=1.0,
                in1=zc[:],
                op0=MULT,
                op1=MULT,
                accum_out=s6acc[:, bk : bk + 1],
            )
        for bk in range(NB_VEC, NB):
            zsq = scr.tile([128, 2 * DY], bf16, tag="zsq", name="zsq")
            nc.scalar.activation(
                zsq[:], zb[bk][:], SQ, accum_out=s6acc[:, bk : bk + 1]
            )

        # two output DMAs on the idle sync ring, in drain-completion order
        # so the second's completion latency hides behind the first's.
        nc.sync.dma_start(fb_out, s6acc[:, 0:NB_VEC])
        nc.sync.dma_start(fa_out, s6acc[:, NB_VEC:NB])

    nc.compile()
    return nc


def _get_program():
    global _PROG
    if _PROG is None:
        _PROG = _build_program()
    return _PROG


_LAST_RESULTS = None


def kernel(noises: np.ndarray, images: np.ndarray) -> np.ndarray:
    from concourse import bass_utils

    global _LAST_RESULTS

    nc = _get_program()

    X = np.ascontiguousarray(images, dtype=np.float32).reshape(N, -1)
    Y = np.ascontiguousarray(noises, dtype=np.float32)

    # host-exact small terms (all O(N*d))
    a = np.einsum("ij,ij->i", X, X, dtype=np.float32)
    b = np.einsum("ij,ij->i", Y, Y, dtype=np.float32)
    ones = np.ones(N, dtype=np.float32)
    H = Y.T @ np.stack([a, ones], axis=1)            # [DY, 2] fp32 sgemm
    G = np.stack([b, ones], axis=1).T @ X            # [2, DX] fp32 sgemm
    a64 = a.astype(np.float64)
    b64 = b.astype(np.float64)
    S1 = a64.sum()
    S2 = b64.sum()
    S3 = a64 @ b64
    S4 = float(H[:, 0].astype(np.float64) @ H[:, 1].astype(np.float64))
    S5 = float(G[0].astype(np.float64) @ G[1].astype(np.float64))

    # device inputs: fp8, DoubleRow pair-interleaved partition-major
    x8 = X.astype(ml_dtypes.float8_e4m3)
    y8 = np.ascontiguousarray(
        Y.astype(ml_dtypes.float8_e4m3).reshape(Q, 2, 128, DY).transpose(2, 0, 1, 3)
    )
    in_maps = []
    for c in range(NCORES):
        xcore = np.ascontiguousarray(
            x8[:, c * KC : (c + 1) * KC].reshape(Q, 2, 128, KC).transpose(2, 0, 1, 3)
        )
        in_maps.append({"x": xcore, "y8": y8})

    res = bass_utils.run_bass_kernel_spmd(nc, in_maps, core_ids=list(range(NCORES)))
    _LAST_RESULTS = res

    S6 = 0.0
    for c in range(NCORES):
        S6 += np.asarray(res.results[c]["fa"], dtype=np.float64).sum()
        S6 += np.asarray(res.results[c]["fb"], dtype=np.float64).sum()

    num = 2.0 * N * S3 + 2.0 * S1 * S2 - 4.0 * S4 - 4.0 * S5 + 4.0 * S6
    mean = num / (float(N) * N * DX * DY)
    return np.asarray(np.exp(-mean), dtype=np.float32)
